# revision 20
# baseline (speedup 1.0000x reference)
"""BiCutLoss Trainium2 kernel (nn_BiCutLoss_52312701665760), v5 (fp16, fused rows).

Reference computation (per batch row i of output[B, L, 2], labels[B, L]):
  temp = argmax(output, -1)            # 1 iff out1 > out0
  cut  = L if all(temp == 1) else (index of last 0 in temp)
  mask = arange(L) < cut
  r1   = where(labels == 1, -3.6/log2(j+2), 0.065)
  loss = sum(out1 * mask * r1) / B

Kernel formulation (equivalent up to fp16 / exact-tie rounding):
  d[j]  = out0[j] - out1[j]                # temp[j]==0  <=>  d[j] >= 0
  M'[j] = max(d[j:] cup {0})               # floor-0 reverse cummax
  thr   = -1 if M'[0] == 0 else 0          # all-ones row => mask all 1
  mask[j] = (M'[j+1] > thr)
  loss_i = sum_j out1*mask*(0.065 + lab*pre2),  pre2[j] = -3.6/log2(j+2)-0.065

Sharding: pure data parallel - B=4096 rows split as 512 rows x 8 cores. Each
core packs TWO rows per SBUF partition (free dim 2L=8192) in 2 super-tiles of
256 rows; host sums the per-partition partials and divides by B.

The scan handles the packed rows in ONE instruction via a multiplicative
reset vector:  state = (rst[t] * state) max d[t], with rst = 0 at each row's
last column (first visited in the reversed scan), restarting the recurrence
at the row boundary.  The floor-0 / strict-> mask convention makes the reset
value (0) the natural pad.  Boundary columns (4095, 8191) are zeroed in w
instead of masked exactly: their true mask is 0 unless the row is all-ones
(probability ~2^-4096 per row), so the contribution is 0 either way.

Perf notes (HW-microbenched sustained costs, [128,4096] op, fp16):
  DVE TT 2x ~2.4-2.8us, STT+accum 2x ~2.6us (needs 4B-aligned APs: the scan
  output is written at +1 element so in0 = M'[j+1] is aligned), scan 1x
  ~5.8us, TS ~1.3us; TS+accum falls to 1x - avoid.  ACT activation ~1.3us.
  Pool TT ~6.8us and contends with DVE for SBUF ports - moving any TT to
  Pool measured SLOWER end-to-end (104 -> 117-146us); all tensor-wide ops
  stay on DVE.  DVE sustained rate is ~1.4x its streaming time (pipe-drain
  between ops), so total DVE STREAM time is the metric that matters; op
  granularity changes (4x4096 vs 2x8192) measured within noise of equal.
  Per iteration: DVE stream = sub 8.7 + lp 8.7 + scan 17.2 + w 8.7 +
  STT 9.0 + tiny ~ 54us -> ~75us sustained.  ACT (cast+rr ~5.4us) and DMA
  (10.5MB HWDGE ~30us) fully hidden.  Measured ~75-79us/iter end-to-end
  (was ~128us for the f32 v1 baseline); fp32 reference rel err 6.4e-5.
"""

import os
from contextlib import ExitStack

import numpy as np

B, L = 4096, 4096
L2 = 2 * L                            # free dim: two rows per partition
N_CORES = 8
ROWS_PER_CORE = B // N_CORES          # 512
P = 128                               # partitions per tile
TILES = 2                             # super-tiles per core (256 rows each)
C_CONST = 0.65 * 0.1                  # 0.065

_CACHE = {}


def _build_nc(repeat: int = 1):
    import concourse.mybir as mybir
    import concourse.tile as tile
    from concourse import bacc

    f16 = mybir.dt.float16
    f32 = mybir.dt.float32
    u8 = mybir.dt.uint8
    Op = mybir.AluOpType
    Act = mybir.ActivationFunctionType

    nc = bacc.Bacc("TRN2", target_bir_lowering=False, debug=False)

    # cat: per partition-row, [t0(rowA) t0(rowB) | t1(rowA) t1(rowB)]
    cat_d = nc.dram_tensor("cat", [TILES * P, 2 * L2], f16, kind="ExternalInput")
    lab_d = nc.dram_tensor("lab", [TILES * P, L2], u8, kind="ExternalInput")
    pre_d = nc.dram_tensor("pre", [P, L2], f16, kind="ExternalInput")
    rst_d = nc.dram_tensor("rst", [P, L2], f16, kind="ExternalInput")
    res_d = nc.dram_tensor("res", [P, 1], f32, kind="ExternalOutput")

    cat_t = cat_d[:].rearrange("(n p) m -> n p m", p=P)   # [2, 128, 16384]
    lab_t = lab_d[:].rearrange("(n p) m -> n p m", p=P)   # [2, 128, 8192]

    with tile.TileContext(nc) as tc, ExitStack() as ctx:
        io_pool = ctx.enter_context(tc.tile_pool(name="io", bufs=2))
        lt_pool = ctx.enter_context(tc.tile_pool(name="ltp", bufs=1))
        cst_pool = ctx.enter_context(tc.tile_pool(name="cst", bufs=1))
        labh_pool = ctx.enter_context(tc.tile_pool(name="labh", bufs=1))
        lp_pool = ctx.enter_context(tc.tile_pool(name="lp", bufs=1))
        rr_pool = ctx.enter_context(tc.tile_pool(name="rr", bufs=1))
        d_pool = ctx.enter_context(tc.tile_pool(name="d", bufs=1))
        m_pool = ctx.enter_context(tc.tile_pool(name="m", bufs=1))
        w_pool = ctx.enter_context(tc.tile_pool(name="w", bufs=1))
        acc_pool = ctx.enter_context(tc.tile_pool(name="acc", bufs=1))

        pre_tl = cst_pool.tile([P, L2], f16)
        nc.sync.dma_start(pre_tl[:], pre_d[:])
        rst_tl = cst_pool.tile([P, L2], f16)
        nc.sync.dma_start(rst_tl[:], rst_d[:])

        acc_B = acc_pool.tile([P, 2 * TILES], f32)

        for _r in range(repeat):
            for k in range(TILES):
                ct = io_pool.tile([P, 2 * L2], f16, tag="ct")
                nc.sync.dma_start(ct[:], cat_t[k])
                lt = lt_pool.tile([P, L2], u8, tag="lt")
                nc.scalar.dma_start(lt[:], lab_t[k])

                t0 = ct[:, 0:L2]
                t1 = ct[:, L2:2 * L2]

                # labels u8 -> f16 (ACT); rr = 0.065 + lab * pre2
                labh = labh_pool.tile([P, L2], f16)
                nc.scalar.activation(labh[:], lt[:], Act.Copy, bias=0.0, scale=1.0)

                # d = t0 - t1 (DVE TT, 2x).  NOTE: offloading any TT to Pool
                # measured consistently SLOWER end-to-end (port contention /
                # scheduling), despite Pool being idle - keep everything DVE.
                d = d_pool.tile([P, L2], f16)
                nc.vector.tensor_tensor(d[:], t0, t1, Op.subtract)

                # lp = lab * pre2 (DVE TT, 2x); rr (ACT) overlaps the scan
                lp = lp_pool.tile([P, L2], f16)
                nc.vector.tensor_tensor(lp[:], labh[:], pre_tl[:], Op.mult)
                rr = rr_pool.tile([P, L2], f16)
                nc.scalar.activation(rr[:], lp[:], Act.Copy, bias=C_CONST, scale=1.0)

                # M'[j] = max(d[j:] cup {0}) per packed row, both rows in one
                # scan: state = (rst*state) max d, rst=0 at row boundaries.
                # Written at +1 element (Mbuf[i+1] = M'[i]) so the STT's
                # in0 = M'[j+1] = Mbuf[j+2] is 4-byte aligned (2x mode).
                M = m_pool.tile([P, L2 + 2], f16)
                nc.vector.memset(M[:, L2 + 1:L2 + 2], 0.0)
                nc.vector.tensor_tensor_scan(
                    M[:, 1:L2 + 1][:, ::-1], rst_tl[:, ::-1], d[:, ::-1], 0.0,
                    Op.mult, Op.max,
                )

                # thr = -1 if M'[row0] == 0 else 0 (all-ones row), per row
                thrA = acc_pool.tile([P, 1], f32, tag="thrA")
                nc.vector.tensor_scalar(
                    thrA[:], M[:, 1:2], 0.0, -1.0, Op.is_le, Op.mult)
                thrB = acc_pool.tile([P, 1], f32, tag="thrB")
                nc.vector.tensor_scalar(
                    thrB[:], M[:, L + 1:L + 2], 0.0, -1.0, Op.is_le, Op.mult)

                # w = t1 * rr (DVE TT, 2x); zero the two boundary columns
                w = w_pool.tile([P, L2], f16)
                nc.vector.tensor_tensor(w[:], t1, rr[:], Op.mult)
                nc.vector.memset(w[:, L - 1:L], 0.0)
                nc.vector.memset(w[:, L2 - 1:L2], 0.0)

                # z = (M'[j+1] > thr) * w per row, accum row-sums (STT, 2x)
                nc.vector.scalar_tensor_tensor(
                    w[:, 0:L], M[:, 2:L + 2], thrA[:], w[:, 0:L],
                    Op.is_gt, Op.mult,
                    accum_out=acc_B[:, 2 * k:2 * k + 1],
                )
                nc.vector.scalar_tensor_tensor(
                    w[:, L:L2], M[:, L + 2:L2 + 2], thrB[:], w[:, L:L2],
                    Op.is_gt, Op.mult,
                    accum_out=acc_B[:, 2 * k + 1:2 * k + 2],
                )

            # tail: loss_i = sum_k loss_k
            loss_t = acc_pool.tile([P, 1], f32, tag="loss")
            nc.vector.reduce_sum(loss_t[:], acc_B[:], axis=mybir.AxisListType.X)

        nc.sync.dma_start(res_d[:], loss_t[:])

    nc.compile()
    return nc


def _pre_tile() -> np.ndarray:
    j = np.arange(L, dtype=np.float64)
    pre2 = (-3.6 / np.log2(j + 2.0) - C_CONST).astype(np.float16)
    row = np.concatenate([pre2, pre2])
    return np.ascontiguousarray(np.tile(row[None, :], (P, 1)))


def _rst_tile() -> np.ndarray:
    rst = np.ones((P, L2), dtype=np.float16)
    rst[:, L - 1] = 0.0
    rst[:, L2 - 1] = 0.0
    return rst


def _get_nc(repeat: int = 1):
    key = repeat
    if key not in _CACHE:
        _CACHE[key] = _build_nc(repeat=repeat)
    return _CACHE[key]


def make_in_maps(output: np.ndarray, labels: np.ndarray):
    pre = _pre_tile()
    rst = _rst_tile()
    # host marshaling: dtype conversion + layout only
    out16 = output.astype(np.float16)                      # [B, L, 2]
    lab8 = labels.astype(np.uint8)                         # [B, L]
    in_maps = []
    for c in range(N_CORES):
        sl = slice(c * ROWS_PER_CORE, (c + 1) * ROWS_PER_CORE)
        t0 = out16[sl, :, 0]                               # [512, L]
        t1 = out16[sl, :, 1]
        lb = lab8[sl]
        catb, labb = [], []
        for s in range(TILES):
            a = 2 * P * s
            catb.append(np.concatenate(
                [t0[a:a + P], t0[a + P:a + 2 * P],
                 t1[a:a + P], t1[a + P:a + 2 * P]], axis=1))   # [128, 4L]
            labb.append(np.concatenate(
                [lb[a:a + P], lb[a + P:a + 2 * P]], axis=1))   # [128, 2L]
        in_maps.append({
            "cat": np.ascontiguousarray(np.concatenate(catb, axis=0)),
            "lab": np.ascontiguousarray(np.concatenate(labb, axis=0)),
            "pre": pre,
            "rst": rst,
        })
    return in_maps


def kernel(output: np.ndarray, labels: np.ndarray) -> np.ndarray:
    from concourse.bass_utils import run_bass_kernel_spmd

    nc = _get_nc(repeat=1)
    in_maps = make_in_maps(output, labels)
    r = run_bass_kernel_spmd(nc, in_maps, core_ids=list(range(N_CORES)))
    total = 0.0
    for res in r.results:
        total += float(res["res"].astype(np.float64).sum())
    return np.float32(total / B)


if __name__ == "__main__":
    # quick standalone run (full inputs, random)
    rng = np.random.default_rng(0)
    out = rng.standard_normal((B, L, 2)).astype(np.float32)
    lab = rng.integers(0, 2, size=(B, L)).astype(np.int32)
    print("loss:", kernel(out, lab))


# revision 26
# speedup vs baseline: 1.0762x; 1.0762x over previous
"""BiCutLoss Trainium2 kernel (nn_BiCutLoss_52312701665760), v5 (fp16, fused rows).

Reference computation (per batch row i of output[B, L, 2], labels[B, L]):
  temp = argmax(output, -1)            # 1 iff out1 > out0
  cut  = L if all(temp == 1) else (index of last 0 in temp)
  mask = arange(L) < cut
  r1   = where(labels == 1, -3.6/log2(j+2), 0.065)
  loss = sum(out1 * mask * r1) / B

Kernel formulation (equivalent up to fp16 / exact-tie rounding):
  d[j]  = out0[j] - out1[j]                # temp[j]==0  <=>  d[j] >= 0
  M'[j] = max(d[j:] cup {0})               # floor-0 reverse cummax
  thr   = -1 if M'[0] == 0 else 0          # all-ones row => mask all 1
  mask[j] = (M'[j+1] > thr)
  loss_i = sum_j out1*mask*(0.065 + lab*pre2),  pre2[j] = -3.6/log2(j+2)-0.065

Sharding: pure data parallel - B=4096 rows split as 512 rows x 8 cores. Each
core packs TWO rows per SBUF partition (free dim 2L=8192) in 2 super-tiles of
256 rows; host sums the per-partition partials and divides by B.

The scan handles the packed rows in ONE instruction via a multiplicative
reset vector:  state = (rst[t] * state) max d[t], with rst = 0 at each row's
last column (first visited in the reversed scan), restarting the recurrence
at the row boundary.  The floor-0 / strict-> mask convention makes the reset
value (0) the natural pad.  Boundary columns (4095, 8191) are zeroed in w
instead of masked exactly: their true mask is 0 unless the row is all-ones
(probability ~2^-4096 per row), so the contribution is 0 either way.

Perf notes (HW-microbenched sustained costs, [128,4096] op, fp16):
  DVE TT 2x ~2.4-2.8us, STT+accum 2x ~2.6us (needs 4B-aligned APs: the scan
  output is written at +1 element so in0 = M'[j+1] is aligned), scan 1x
  ~5.8us, TS ~1.3us; TS+accum falls to 1x - avoid.  ACT activation ~1.3us.
  Pool TT ~6.8us and contends with DVE for SBUF ports - moving any TT to
  Pool measured SLOWER end-to-end (104 -> 117-146us); all tensor-wide ops
  stay on DVE.  DVE sustained rate is ~1.4x its streaming time (pipe-drain
  between ops), so total DVE STREAM time is the metric that matters; op
  granularity changes (4x4096 vs 2x8192) measured within noise of equal.
  Per iteration: DVE stream = sub 8.7 + lp 8.7 + scan 17.2 + w 8.7 +
  STT 9.0 + tiny ~ 54us -> ~75us sustained.  ACT (cast+rr ~5.4us) and DMA
  (10.5MB HWDGE ~30us) fully hidden.  Measured ~75-79us/iter end-to-end
  (was ~128us for the f32 v1 baseline); fp32 reference rel err 6.4e-5.
"""

import os
from contextlib import ExitStack

import numpy as np

B, L = 4096, 4096
L2 = 2 * L                            # free dim: two rows per partition
N_CORES = 8
ROWS_PER_CORE = B // N_CORES          # 512
P = 128                               # partitions per tile
TILES = 2                             # super-tiles per core (256 rows each)
C_CONST = 0.65 * 0.1                  # 0.065

_CACHE = {}


def _build_nc(repeat: int = 1):
    import concourse.mybir as mybir
    import concourse.tile as tile
    from concourse import bacc

    f16 = mybir.dt.float16
    f32 = mybir.dt.float32
    u8 = mybir.dt.uint8
    Op = mybir.AluOpType
    Act = mybir.ActivationFunctionType

    nc = bacc.Bacc("TRN2", target_bir_lowering=False, debug=False)

    # cat: per partition-row, [t0(rowA) t0(rowB) | t1(rowA) t1(rowB)]
    cat_d = nc.dram_tensor("cat", [TILES * P, 2 * L2], f16, kind="ExternalInput")
    lab_d = nc.dram_tensor("lab", [TILES * P, L2], u8, kind="ExternalInput")
    pre_d = nc.dram_tensor("pre", [P, L2], f16, kind="ExternalInput")
    rst_d = nc.dram_tensor("rst", [P, L2], f16, kind="ExternalInput")
    res_d = nc.dram_tensor("res", [P, 1], f32, kind="ExternalOutput")

    cat_t = cat_d[:].rearrange("(n p) m -> n p m", p=P)   # [2, 128, 16384]
    lab_t = lab_d[:].rearrange("(n p) m -> n p m", p=P)   # [2, 128, 8192]

    with tile.TileContext(nc) as tc, ExitStack() as ctx:
        io_pool = ctx.enter_context(tc.tile_pool(name="io", bufs=2))
        lt_pool = ctx.enter_context(tc.tile_pool(name="ltp", bufs=1))
        cst_pool = ctx.enter_context(tc.tile_pool(name="cst", bufs=1))
        labh_pool = ctx.enter_context(tc.tile_pool(name="labh", bufs=1))
        lp_pool = ctx.enter_context(tc.tile_pool(name="lp", bufs=1))
        rr_pool = ctx.enter_context(tc.tile_pool(name="rr", bufs=1))
        d_pool = ctx.enter_context(tc.tile_pool(name="d", bufs=1))
        m_pool = ctx.enter_context(tc.tile_pool(name="m", bufs=1))
        w_pool = ctx.enter_context(tc.tile_pool(name="w", bufs=1))
        acc_pool = ctx.enter_context(tc.tile_pool(name="acc", bufs=1))

        pre_tl = cst_pool.tile([P, L2], f16)
        nc.sync.dma_start(pre_tl[:], pre_d[:])
        rst_tl = cst_pool.tile([P, L2], f16)
        nc.sync.dma_start(rst_tl[:], rst_d[:])

        acc_B = acc_pool.tile([P, 2 * TILES], f32)

        for _r in range(repeat):
            for k in range(TILES):
                ct = io_pool.tile([P, 2 * L2], f16, tag="ct")
                nc.sync.dma_start(ct[:], cat_t[k])
                lt = lt_pool.tile([P, L2], u8, tag="lt")
                nc.scalar.dma_start(lt[:], lab_t[k])

                t0 = ct[:, 0:L2]
                t1 = ct[:, L2:2 * L2]

                # labels u8 -> f16 (ACT); rr = 0.065 + lab * pre2
                labh = labh_pool.tile([P, L2], f16)
                nc.scalar.activation(labh[:], lt[:], Act.Copy, bias=0.0, scale=1.0)

                # d = t0 - t1 (DVE TT, 2x).  NOTE: offloading any TT to Pool
                # measured consistently SLOWER end-to-end (port contention /
                # scheduling), despite Pool being idle - keep everything DVE.
                d = d_pool.tile([P, L2], f16)
                nc.vector.tensor_tensor(d[:], t0, t1, Op.subtract)

                # lp = lab * pre2 (DVE TT, 2x); rr (ACT) overlaps the scan
                lp = lp_pool.tile([P, L2], f16)
                nc.vector.tensor_tensor(lp[:], labh[:], pre_tl[:], Op.mult)
                rr = rr_pool.tile([P, L2], f16)
                nc.scalar.activation(rr[:], lp[:], Act.Copy, bias=C_CONST, scale=1.0)

                # M'[j] = max(d[j:] cup {0}) per packed row, both rows in one
                # scan: state = (rst*state) max d, rst=0 at row boundaries.
                # Written at +1 element (Mbuf[i+1] = M'[i]) so the STT's
                # in0 = M'[j+1] = Mbuf[j+2] is 4-byte aligned (2x mode).
                M = m_pool.tile([P, L2 + 2], f16)
                nc.vector.memset(M[:, L2 + 1:L2 + 2], 0.0)
                nc.vector.tensor_tensor_scan(
                    M[:, 1:L2 + 1][:, ::-1], rst_tl[:, ::-1], d[:, ::-1], 0.0,
                    Op.mult, Op.max,
                )

                # thr = -1 if M'[row0] == 0 else 0 (all-ones row), per row
                thrA = acc_pool.tile([P, 1], f32, tag="thrA")
                nc.vector.tensor_scalar(
                    thrA[:], M[:, 1:2], 0.0, -1.0, Op.is_le, Op.mult)
                thrB = acc_pool.tile([P, 1], f32, tag="thrB")
                nc.vector.tensor_scalar(
                    thrB[:], M[:, L + 1:L + 2], 0.0, -1.0, Op.is_le, Op.mult)

                # w = t1 * rr (DVE TT, 2x); zero the two boundary columns
                # (contiguous memsets on DVE: strided APs and cross-engine
                # tiny ops both measured slower)
                w = w_pool.tile([P, L2], f16)
                nc.vector.tensor_tensor(w[:], t1, rr[:], Op.mult)
                nc.vector.memset(w[:, L - 1:L], 0.0)
                nc.vector.memset(w[:, L2 - 1:L2], 0.0)

                # z = (M'[j+1] > thr) * w per row, accum row-sums (STT, 2x)
                nc.vector.scalar_tensor_tensor(
                    w[:, 0:L], M[:, 2:L + 2], thrA[:], w[:, 0:L],
                    Op.is_gt, Op.mult,
                    accum_out=acc_B[:, 2 * k:2 * k + 1],
                )
                nc.vector.scalar_tensor_tensor(
                    w[:, L:L2], M[:, L + 2:L2 + 2], thrB[:], w[:, L:L2],
                    Op.is_gt, Op.mult,
                    accum_out=acc_B[:, 2 * k + 1:2 * k + 2],
                )

            # tail: loss_i = sum_k loss_k
            loss_t = acc_pool.tile([P, 1], f32, tag="loss")
            nc.vector.reduce_sum(loss_t[:], acc_B[:], axis=mybir.AxisListType.X)

        nc.sync.dma_start(res_d[:], loss_t[:])

    nc.compile()
    return nc


def _pre_tile() -> np.ndarray:
    j = np.arange(L, dtype=np.float64)
    pre2 = (-3.6 / np.log2(j + 2.0) - C_CONST).astype(np.float16)
    row = np.concatenate([pre2, pre2])
    return np.ascontiguousarray(np.tile(row[None, :], (P, 1)))


def _rst_tile() -> np.ndarray:
    rst = np.ones((P, L2), dtype=np.float16)
    rst[:, L - 1] = 0.0
    rst[:, L2 - 1] = 0.0
    return rst


def _get_nc(repeat: int = 1):
    key = repeat
    if key not in _CACHE:
        _CACHE[key] = _build_nc(repeat=repeat)
    return _CACHE[key]


def make_in_maps(output: np.ndarray, labels: np.ndarray):
    pre = _pre_tile()
    rst = _rst_tile()
    # host marshaling: dtype conversion + layout only
    out16 = output.astype(np.float16)                      # [B, L, 2]
    lab8 = labels.astype(np.uint8)                         # [B, L]
    in_maps = []
    for c in range(N_CORES):
        sl = slice(c * ROWS_PER_CORE, (c + 1) * ROWS_PER_CORE)
        t0 = out16[sl, :, 0]                               # [512, L]
        t1 = out16[sl, :, 1]
        lb = lab8[sl]
        catb, labb = [], []
        for s in range(TILES):
            a = 2 * P * s
            catb.append(np.concatenate(
                [t0[a:a + P], t0[a + P:a + 2 * P],
                 t1[a:a + P], t1[a + P:a + 2 * P]], axis=1))   # [128, 4L]
            labb.append(np.concatenate(
                [lb[a:a + P], lb[a + P:a + 2 * P]], axis=1))   # [128, 2L]
        in_maps.append({
            "cat": np.ascontiguousarray(np.concatenate(catb, axis=0)),
            "lab": np.ascontiguousarray(np.concatenate(labb, axis=0)),
            "pre": pre,
            "rst": rst,
        })
    return in_maps


def kernel(output: np.ndarray, labels: np.ndarray) -> np.ndarray:
    from concourse.bass_utils import run_bass_kernel_spmd

    nc = _get_nc(repeat=1)
    in_maps = make_in_maps(output, labels)
    r = run_bass_kernel_spmd(nc, in_maps, core_ids=list(range(N_CORES)))
    total = 0.0
    for res in r.results:
        total += float(res["res"].astype(np.float64).sum())
    return np.float32(total / B)


if __name__ == "__main__":
    # quick standalone run (full inputs, random)
    rng = np.random.default_rng(0)
    out = rng.standard_normal((B, L, 2)).astype(np.float32)
    lab = rng.integers(0, 2, size=(B, L)).astype(np.int32)
    print("loss:", kernel(out, lab))
